# revision 32
# baseline (speedup 1.0000x reference)
"""CacheAwareMHA TRN2 kernel: 8-core head-sharded attention (bf16 edition).

Strategy:
  - Shard heads (16) across 8 cores: 2 heads/core. Each core holds W_q/W_o
    column/row shards and its heads' K/V slices; partial outputs summed on host.
  - All matmul operands bf16 (full PE rate like fp32r, but half the DMA bytes
    and no small-N rate cliff); PSUM accumulation stays fp32.
  - S^T layout [m partitions, t free] so softmax weights feed P@V directly as
    matmul operands with no on-device transposes anywhere.
  - Queries sorted by position on host -> causal mask is a per-(m-tile) t-prefix;
    fully-masked columns are skipped via per-m-tile column windows (S/exp/PV/
    rowsum all windowed); the ~128-position partial band is masked with one
    scalar_tensor_tensor (pos >= m) * P per m-tile.
  - exp without max-subtraction (logits ~N(0,1); overflow impossible).
  - Row sums via ones-matmul on PE (replicated), reciprocal_approx_fast, fold
    into O^T normalize; out = W_o-shard projection in bf16, partials summed on
    host in fp64.
  - xT staged in DRAM as [128, t-half, ko, 512] so Q projection for the first
    t-half starts after ~2MB instead of the full 4.2MB.
"""
import sys
import math

import numpy as np

for _p in ("/opt/trn_rl_repo", "/opt/pypackages"):
    if _p not in sys.path:
        sys.path.append(_p)

import ml_dtypes

BF16 = ml_dtypes.bfloat16

T, D, H, DK, M = 1024, 2048, 16, 128, 4096
NCORES = 8
HLOC = H // NCORES  # heads per core
KO = D // 128       # 16 contraction tiles for projections
MT = M // 128       # 32 m-tiles
ROPE_BASE = 10000.0
SCALE = 1.0 / math.sqrt(DK)

_PROGRAM_CACHE = {}


def _host_rope_k(k, pos):
    """Apply RoPE to cached keys on host (fp64 tables). k: [M, h, DK]."""
    inv = 1.0 / (ROPE_BASE ** (np.arange(0, DK, 2, dtype=np.float64) / DK))
    th = pos[:, None].astype(np.float64) * inv[None, :]
    cos = np.concatenate([np.cos(th), np.cos(th)], -1)[:, None, :]
    sin = np.concatenate([np.sin(th), np.sin(th)], -1)[:, None, :]
    t1, t2 = k[..., :64], k[..., 64:]
    rot = np.concatenate([-t2, t1], -1)
    return (k.astype(np.float64) * cos + rot.astype(np.float64) * sin).astype(np.float32)


def _host_q_tables(pos_sorted):
    """cos / sign-baked sin tables in Q^T layout [DK, T] (fp32)."""
    inv = 1.0 / (ROPE_BASE ** (np.arange(0, DK, 2, dtype=np.float64) / DK))
    th = pos_sorted[None, :].astype(np.float64) * inv[:, None]      # [64, T]
    cos = np.cos(th)
    sin = np.sin(th)
    cosT = np.concatenate([cos, cos], 0).astype(np.float32)          # [128, T]
    sinT = np.concatenate([-sin, sin], 0).astype(np.float32)         # sign baked
    return cosT, sinT


RSG = 4  # m-tiles per rowsum group (P-tiles pre-summed on DVE, one ones-matmul)


def _windows(a_list, b_list):
    """Per m-tile: chunk column windows + exp/stt windows.

    Returns (tiles, groups). tiles[i] = (chunk_lo[2], exp_lo, stt_lo, stt_hi):
    chunk_lo[c] is the start column for S/PV matmuls in chunk c (None = skip);
    starts are 16-col aligned (32B for bf16 matmul operands). exp/stt start at
    the GROUP's min window so grouped P-tiles can be tree-summed over one
    region (the widened prefix is exactly zeroed by the stt mask).
    groups[g] = (glo_exp, rs_lo[2]): the group's exp window start and per-chunk
    rowsum matmul windows."""
    raw = []
    for i in range(MT):
        a = a_list[i]
        chunk_lo = []
        for c in range(2):
            lo_raw = max(512 * c, a)
            hi = 512 * (c + 1)
            if hi - lo_raw <= 0:
                chunk_lo.append(None)
                continue
            chunk_lo.append(max(512 * c, lo_raw & ~15))
        raw.append(chunk_lo)
    groups = []
    for g in range(MT // RSG):
        members = raw[g * RSG:(g + 1) * RSG]
        rs_lo = []
        for c in range(2):
            los = [m[c] for m in members if m[c] is not None]
            rs_lo.append(min(los) if los else None)
        glo_exp = next((rs_lo[c] for c in range(2) if rs_lo[c] is not None), None)
        groups.append((glo_exp, rs_lo))
    tiles = []
    for i in range(MT):
        chunk_lo = raw[i]
        glo_exp = groups[i // RSG][0]
        tiles.append((chunk_lo, glo_exp, glo_exp, b_list[i]))
    return tiles, groups


def _build_program(a_list, b_list):
    """Build the single-core Bass program (same for all cores)."""
    import concourse.tile as tile
    import concourse.mybir as mybir
    from concourse import bacc
    from contextlib import ExitStack

    f32 = mybir.dt.float32
    f32r = mybir.dt.float32r
    bf16 = mybir.dt.bfloat16
    win, grp = _windows(a_list, b_list)

    nc = bacc.Bacc("TRN2", target_bir_lowering=False, debug=False, num_devices=NCORES)

    d_xT = nc.dram_tensor("xT", (128, 2, KO, 512), bf16, kind="ExternalInput").ap()
    d_wqT = nc.dram_tensor("wqT", (128, KO, HLOC * DK), bf16, kind="ExternalInput").ap()
    d_ktr = nc.dram_tensor("ktr", (HLOC, DK, M), bf16, kind="ExternalInput").ap()
    d_v = nc.dram_tensor("v", (HLOC, 128, MT, DK), bf16, kind="ExternalInput").ap()
    d_woT = nc.dram_tensor("woT", (128, HLOC, D), bf16, kind="ExternalInput").ap()
    d_cosq = nc.dram_tensor("cosq", (DK, T), f32, kind="ExternalInput").ap()
    d_sinq = nc.dram_tensor("sinq", (DK, T), f32, kind="ExternalInput").ap()
    d_posr = nc.dram_tensor("posr", (128, T), f32r, kind="ExternalInput").ap()
    d_miota = nc.dram_tensor("miota", (128, MT), f32r, kind="ExternalInput").ap()
    d_ones = nc.dram_tensor("ones", (128, 128), bf16, kind="ExternalInput").ap()
    d_out = nc.dram_tensor("outT", (D, T), bf16, kind="ExternalOutput").ap()

    with tile.TileContext(nc) as tc, ExitStack() as ctx:
        const = ctx.enter_context(tc.tile_pool(name="const", bufs=1))
        big = ctx.enter_context(tc.tile_pool(name="big", bufs=1))
        qpool = ctx.enter_context(tc.tile_pool(name="qpool", bufs=2))
        qtmp = ctx.enter_context(tc.tile_pool(name="qtmp", bufs=2))
        ps_main = ctx.enter_context(tc.tile_pool(name="ps_main", bufs=2, space="PSUM"))
        ps_acc = ctx.enter_context(tc.tile_pool(name="ps_acc", bufs=1, space="PSUM"))
        ps_rs = ctx.enter_context(tc.tile_pool(name="ps_rs", bufs=1, space="PSUM"))
        xpool_cm = tc.tile_pool(name="xpool", bufs=1)
        xpool = xpool_cm.__enter__()

        # ---------------- loads ----------------
        # sync ring order IS the schedule: tiny mask tables first, then the
        # Qproj critical path (wqT, xT by t-half), then K/V/woT behind it.
        # Scalar ring carries only the rope tables so the ACT queue is never
        # blocked by slow DMA descriptor generation (e.g. posr's broadcast).
        ones_sb = const.tile([128, 128], bf16, name="ones_sb")
        nc.scalar.dma_start(out=ones_sb[:], in_=d_ones)
        wqT_sb = xpool.tile([128, KO, HLOC * DK], bf16, name="wqT_sb")
        nc.sync.dma_start(out=wqT_sb[:], in_=d_wqT)
        xT_sb = xpool.tile([128, 2, KO, 512], bf16, name="xT_sb")
        for c in range(2):
            for g in range(4):
                sl = slice(g * 4, (g + 1) * 4)
                nc.sync.dma_start(out=xT_sb[:, c, sl, :], in_=d_xT[:, c, sl, :])
        cosq_sb = const.tile([128, T], f32, name="cosq_sb")
        nc.scalar.dma_start(out=cosq_sb[:], in_=d_cosq)
        sinq_sb = const.tile([128, T], f32, name="sinq_sb")
        nc.scalar.dma_start(out=sinq_sb[:], in_=d_sinq)
        posr_sb = const.tile([128, T], f32r, name="posr_sb")
        nc.scalar.dma_start(out=posr_sb[:], in_=d_posr)
        miota_sb = const.tile([128, MT], f32r, name="miota_sb")
        nc.scalar.dma_start(out=miota_sb[:], in_=d_miota)

        # bulk loads behind xT on the sync ring: ktr0/v0 land just before the
        # first S/PV needs them
        ktr_sb = []
        v_sb = []
        for h in range(HLOC):
            kt = big.tile([128, M], bf16, name=f"ktr_sb{h}")
            nc.sync.dma_start(out=kt[:], in_=d_ktr[h])
            ktr_sb.append(kt)
            vt = big.tile([128, MT, DK], bf16, name=f"v_sb{h}")
            nc.sync.dma_start(out=vt[:], in_=d_v[h])
            v_sb.append(vt)
        woT_sb = big.tile([128, HLOC, D], bf16, name="woT_sb")
        nc.sync.dma_start(out=woT_sb[:], in_=d_woT)

        # dummy exp: forces the ACT exp-table load off the critical path
        # (it happens here, while ACT is otherwise idle during the loads)
        warm = const.tile([128, 16], bf16, name="warm")
        nc.scalar.activation(warm[:], ones_sb[:, 0:16],
                             mybir.ActivationFunctionType.Exp)

        # ---------------- Q projection + RoPE ----------------
        # h-outer: head 0's both chunks project+rope first, so the attention
        # stream (which needs full qtr[0]) starts as early as possible
        qps = []
        qtr = []
        for h in range(HLOC):
            # allocate Qproj accumulators from the attention accumulator pools
            # (idle during Qproj) so ps_main's S-buffers stay free
            qpool_ps = ps_acc if h == 0 else ps_rs
            qtag = "oacc" if h == 0 else "rs"
            qps.append(qpool_ps.tile([128, T], f32, tag=qtag, name=f"qps{h}"))
            qtr.append(qpool.tile([128, T], bf16, tag="qtr", name=f"qtr{h}"))
        # chain order: head 0's both chunks first (its rope gates the whole
        # attention stream); each rope is emitted right after its producing
        # chain — tile deps are tracked conservatively in program order, so
        # a rope emitted later would wait on unrelated writes to its qps tile
        for c, h in [(0, 0), (0, 1), (1, 0), (1, 1)]:
            cs = slice(c * 512, (c + 1) * 512)
            for k in range(KO):
                nc.tensor.matmul(
                    qps[h][:, cs],
                    wqT_sb[:, k, h * DK:(h + 1) * DK],
                    xT_sb[:, c, k, :],
                    start=(k == 0), stop=(k == KO - 1),
                )
            qrot = qtmp.tile([128, 512], f32, tag="qrot")
            nc.vector.tensor_copy(qrot[0:64, :], qps[h][64:128, cs])
            nc.vector.tensor_copy(qrot[64:128, :], qps[h][0:64, cs])
            t1 = qtmp.tile([128, 512], f32, tag="t1")
            nc.vector.tensor_mul(t1[:], qrot[:], sinq_sb[:, cs])
            t2 = qtmp.tile([128, 512], f32, tag="t2")
            nc.vector.tensor_mul(t2[:], qps[h][:, cs], cosq_sb[:, cs])
            nc.vector.tensor_add(qtr[h][:, cs], t1[:], t2[:])

        xpool_cm.__exit__(None, None, None)  # free xT/wqT SBUF for attention pools
        ppool = ctx.enter_context(tc.tile_pool(name="ppool", bufs=6))
        gpool = ctx.enter_context(tc.tile_pool(name="gpool", bufs=3))
        opool = ctx.enter_context(tc.tile_pool(name="opool", bufs=2))
        ostage = ctx.enter_context(tc.tile_pool(name="ostage", bufs=3))

        # ---------------- attention per head ----------------
        # PE issue order is software-pipelined: S(i+2) is issued right after
        # PV(i) so exp(i+1) never waits behind PV(i) in the in-order PE queue;
        # rowsum group matmuls are deferred ~2 tiles so their DVE tree-adds
        # are complete by the time the PE reaches them.
        onorm = []
        ptiles = {}

        def emit_s_exp(h, i):
            chunk_lo, exp_lo, stt_lo, stt_hi = win[i]
            s_lo = grp[i // RSG][1]  # group windows: exp reads only S-written psum
            sps = ps_main.tile([128, T], f32, tag="mm", name=f"s_{h}_{i}")
            for c in range(2):
                lo = s_lo[c]
                if lo is None:
                    continue
                nc.tensor.matmul(
                    sps[:, lo:512 * (c + 1)],
                    ktr_sb[h][:, i * 128:(i + 1) * 128],
                    qtr[h][:, lo:512 * (c + 1)],
                    start=True, stop=True,
                )
            p = ppool.tile([128, T], bf16, tag="p")
            nc.scalar.activation(p[:, exp_lo:], sps[:, exp_lo:],
                                 mybir.ActivationFunctionType.Exp, scale=SCALE)
            if stt_hi > stt_lo:
                nc.vector.scalar_tensor_tensor(
                    out=p[:, stt_lo:stt_hi], in0=posr_sb[:, stt_lo:stt_hi],
                    scalar=miota_sb[:, i:i + 1], in1=p[:, stt_lo:stt_hi],
                    op0=mybir.AluOpType.is_ge, op1=mybir.AluOpType.mult,
                )
            ptiles[(h, i)] = p

        # who issues the final (stop) rowsum write per chunk: a direct matmul
        # in the last group if that chunk exists there, else the last tree group
        last_writer = {}
        for c in range(2):
            lg = max((g for g in range(MT // RSG) if grp[g][1][c] is not None),
                     default=None)
            if lg == MT // RSG - 1:
                lk = max(k for k in range(RSG)
                         if win[lg * RSG + k][0][c] is not None)
                last_writer[c] = ("direct", lk)
            else:
                last_writer[c] = ("tree", lg)

        emit_s_exp(0, 0)
        emit_s_exp(0, 1)
        for h in range(HLOC):
            ops_t = ps_acc.tile([128, T], f32, tag="oacc", name=f"oacc{h}")
            rs_t = ps_rs.tile([128, T], f32, tag="rs", name=f"rs{h}")
            started = [False, False]
            rs_started = [False, False]
            gsums = {}
            pending_rs = []
            for i in range(MT):
                chunk_lo, exp_lo, stt_lo, stt_hi = win[i]
                p = ptiles[(h, i)]
                for c in range(2):
                    lo = chunk_lo[c]
                    if lo is None:
                        continue
                    nc.tensor.matmul(
                        ops_t[:, lo:512 * (c + 1)],
                        v_sb[h][:, i, :],
                        p[:, lo:512 * (c + 1)],
                        start=not started[c], stop=(i == MT - 1),
                    )
                    started[c] = True
                if i % RSG == RSG - 1 and i // RSG == MT // RSG - 1:
                    # last group: feed rowsums directly from the P tiles (no
                    # tree) so the normalize isn't gated on trailing DVE adds
                    g = i // RSG
                    for k in range(RSG):
                        tk = g * RSG + k
                        k_lo = win[tk][0]
                        for c in range(2):
                            lo = k_lo[c]
                            if lo is None:
                                continue
                            nc.tensor.matmul(
                                rs_t[:, lo:512 * (c + 1)],
                                ones_sb[:],
                                ptiles[(h, tk)][:, lo:512 * (c + 1)],
                                start=not rs_started[c],
                                stop=(last_writer[c] == ("direct", k)),
                            )
                            rs_started[c] = True
                elif i % RSG == RSG - 1:
                    # tree-sum the group's P tiles on DVE (bf16, SBUF->SBUF)
                    g = i // RSG
                    glo, _ = grp[g]
                    pg = [ptiles[(h, g * RSG + k)] for k in range(RSG)]
                    t01 = gpool.tile([128, T], bf16, tag="t01")
                    nc.vector.tensor_add(t01[:, glo:], pg[0][:, glo:], pg[1][:, glo:])
                    t23 = gpool.tile([128, T], bf16, tag="t23")
                    nc.vector.tensor_add(t23[:, glo:], pg[2][:, glo:], pg[3][:, glo:])
                    psum_g = gpool.tile([128, T], bf16, tag="psum_g")
                    nc.vector.tensor_add(psum_g[:, glo:], t01[:, glo:], t23[:, glo:])
                    gsums[g] = psum_g
                    pending_rs.append(g)
                # deferred rowsum matmuls: group g is safe once PV(4g+6) issued
                while pending_rs and 4 * pending_rs[0] + 6 <= i:
                    g = pending_rs.pop(0)
                    _, rs_lo = grp[g]
                    for c in range(2):
                        lo = rs_lo[c]
                        if lo is None:
                            continue
                        nc.tensor.matmul(
                            rs_t[:, lo:512 * (c + 1)],
                            ones_sb[:],
                            gsums[g][:, lo:512 * (c + 1)],
                            start=not rs_started[c],
                            stop=(last_writer[c] == ("tree", g)),
                        )
                        rs_started[c] = True
                # pipeline 2 S/exp tiles ahead, crossing the head boundary so
                # the next head's exp stream starts before this head's drain
                ni = i + 2
                if ni < MT:
                    emit_s_exp(h, ni)
                elif h + 1 < HLOC:
                    emit_s_exp(h + 1, ni - MT)
            for g in pending_rs:
                _, rs_lo = grp[g]
                for c in range(2):
                    lo = rs_lo[c]
                    if lo is None:
                        continue
                    nc.tensor.matmul(
                        rs_t[:, lo:512 * (c + 1)],
                        ones_sb[:],
                        gsums[g][:, lo:512 * (c + 1)],
                        start=not rs_started[c],
                        stop=(last_writer[c] == ("tree", g)),
                    )
                    rs_started[c] = True
            oh = opool.tile([128, T], bf16, tag=f"onorm{h}", name=f"onorm{h}")
            onorm.append(oh)
            if h == 0:
                for c in range(2):
                    cs = slice(c * 512, (c + 1) * 512)
                    rsinv = qtmp.tile([128, 512], f32, tag="rsinv")
                    nc.vector.reciprocal_approx_fast(out=rsinv[:], in_=rs_t[:, cs])
                    nc.vector.tensor_mul(oh[:, cs], ops_t[:, cs], rsinv[:])
            else:
                last_ops, last_rs = ops_t, rs_t

        # ---------------- output projection ----------------
        # head-0 contributions for j=0,1 are issued (in program order) BEFORE
        # head 1's normalize: they only need onorm[0] (ready mid-kernel), so
        # the PE stays busy while the normalize runs on DVE (avoids an idle
        # gap + p-state drop)
        outT_r = d_out.rearrange("(jo p) t -> p jo t", p=128)
        jtiles = {}
        for j in range(2):
            jps = ps_main.tile([128, T], f32, tag="mm", name=f"jps{j}")
            jtiles[j] = jps
            for c in range(2):
                nc.tensor.matmul(
                    jps[:, c * 512:(c + 1) * 512],
                    woT_sb[:, 0, j * 128:(j + 1) * 128],
                    onorm[0][:, c * 512:(c + 1) * 512],
                    start=True, stop=False,
                )
        for c in range(2):
            cs = slice(c * 512, (c + 1) * 512)
            rsinv = qtmp.tile([128, 512], f32, tag="rsinv")
            nc.vector.reciprocal_approx_fast(out=rsinv[:], in_=last_rs[:, cs])
            nc.vector.tensor_mul(onorm[1][:, cs], last_ops[:, cs], rsinv[:])

        for j in range(KO):
            if j in jtiles:
                jps = jtiles[j]
                hos = [1]
            elif j % 4 == 2:
                jps = ps_acc.tile([128, T], f32, tag="oacc", name=f"jps{j}")
                hos = [0, 1]
            elif j % 4 == 3:
                jps = ps_rs.tile([128, T], f32, tag="rs", name=f"jps{j}")
                hos = [0, 1]
            else:
                jps = ps_main.tile([128, T], f32, tag="mm", name=f"jps{j}")
                hos = [0, 1]
            for ho in hos:
                for c in range(2):
                    nc.tensor.matmul(
                        jps[:, c * 512:(c + 1) * 512],
                        woT_sb[:, ho, j * 128:(j + 1) * 128],
                        onorm[ho][:, c * 512:(c + 1) * 512],
                        start=(ho == 0 and j not in jtiles), stop=(ho == HLOC - 1),
                    )
            ost = ostage.tile([128, T], bf16, tag="ost")
            if j >= KO - 2:
                # drain: split the copy across both engines to halve latency
                nc.vector.tensor_copy(ost[:, 0:512], jps[:, 0:512])
                nc.scalar.copy(ost[:, 512:1024], jps[:, 512:1024])
                nc.sync.dma_start(out=outT_r[:, j, 0:512], in_=ost[:, 0:512])
                nc.scalar.dma_start(out=outT_r[:, j, 512:1024], in_=ost[:, 512:1024])
            elif j % 2 == 0:
                nc.vector.tensor_copy(ost[:], jps[:])
                nc.sync.dma_start(out=outT_r[:, j, :], in_=ost[:])
            else:
                nc.scalar.copy(ost[:], jps[:])
                nc.scalar.dma_start(out=outT_r[:, j, :], in_=ost[:])

    nc.compile()
    return nc


def _prep(inputs):
    """Host-side prep shared by kernel() and test harnesses."""
    x = np.asarray(inputs["x"], dtype=np.float32)
    k_ctx = np.asarray(inputs["k_ctx"], dtype=np.float32)
    v_ctx = np.asarray(inputs["v_ctx"], dtype=np.float32)
    W_q = np.asarray(inputs["W_q"], dtype=np.float32)
    W_o = np.asarray(inputs["W_o"], dtype=np.float32)
    pos_np = np.asarray(inputs["positions"]).astype(np.int64)
    pctx_np = np.asarray(inputs["p_ctx"]).astype(np.int64)

    perm = np.argsort(pos_np, kind="stable")
    ps = pos_np[perm]
    xT = np.ascontiguousarray(
        x[perm].T.reshape(KO, 128, 2, 512).transpose(1, 2, 0, 3)).astype(BF16)
    k_rope = _host_rope_k(k_ctx, pctx_np)
    cosq, sinq = _host_q_tables(ps)
    posr = np.ascontiguousarray(
        np.broadcast_to(ps.astype(np.float32).reshape(1, T), (128, T)))
    miota = (np.arange(MT)[None, :] * 128 + np.arange(128)[:, None]).astype(np.float32)
    ones = np.ones((128, 128), dtype=BF16)
    a_list = [int(np.searchsorted(ps, 128 * i, side="left")) for i in range(MT)]
    b_list = [int(np.searchsorted(ps, 128 * i + 127, side="left")) for i in range(MT)]

    in_maps = []
    for c in range(NCORES):
        hs = slice(c * HLOC * DK, (c + 1) * HLOC * DK)
        heads = range(c * HLOC, (c + 1) * HLOC)
        wq = W_q[hs, :].T.reshape(KO, 128, HLOC * DK)          # [ko, p, o]
        wo = W_o[:, hs].T.reshape(HLOC, 128, D)                 # [ho, p, j]
        vv = v_ctx.transpose(1, 0, 2)[c * HLOC:(c + 1) * HLOC]  # [hloc, M, DK]
        in_maps.append({
            "xT": xT,
            "wqT": np.ascontiguousarray(wq.transpose(1, 0, 2)).astype(BF16),
            "ktr": np.ascontiguousarray(
                np.stack([k_rope[:, h, :].T for h in heads])).astype(BF16),
            "v": np.ascontiguousarray(
                vv.reshape(HLOC, MT, 128, DK).transpose(0, 2, 1, 3)).astype(BF16),
            "woT": np.ascontiguousarray(wo.transpose(1, 0, 2)).astype(BF16),
            "cosq": cosq, "sinq": sinq, "posr": posr,
            "miota": miota, "ones": ones,
        })
    return perm, a_list, b_list, in_maps


def kernel(x, k_ctx, v_ctx, W_q, W_o, positions, p_ctx):
    from concourse.bass_utils import run_bass_kernel_spmd

    inputs = dict(x=x, k_ctx=k_ctx, v_ctx=v_ctx, W_q=W_q, W_o=W_o,
                  positions=positions, p_ctx=p_ctx)
    perm, a_list, b_list, in_maps = _prep(inputs)

    key = (tuple(a_list), tuple(b_list))
    if key not in _PROGRAM_CACHE:
        _PROGRAM_CACHE[key] = _build_program(a_list, b_list)
    nc = _PROGRAM_CACHE[key]

    r = run_bass_kernel_spmd(nc, in_maps, core_ids=list(range(NCORES)))

    acc = np.zeros((D, T), dtype=np.float64)
    for c in range(NCORES):
        acc += r.results[c]["outT"].astype(np.float64)
    out_sorted = acc.T.astype(np.float32)
    out = np.empty_like(out_sorted)
    out[perm] = out_sorted
    return out.astype(np.float32)


if __name__ == "__main__":
    import importlib.util
    spec = importlib.util.spec_from_file_location("reference", "/root/problem/reference.py")
    ref = importlib.util.module_from_spec(spec)
    spec.loader.exec_module(ref)
    inputs = {k: np.asarray(v) for k, v in ref.setup_inputs().items()}
    expected = np.asarray(ref.reference(**inputs))
    got = kernel(**inputs)
    err = np.abs(got - expected)
    print("absmax err:", err.max(), "rel:", err.max() / np.abs(expected).max())


# revision 33
# speedup vs baseline: 1.0289x; 1.0289x over previous
"""CacheAwareMHA TRN2 kernel: 8-core head-sharded attention (bf16 edition).

Strategy:
  - Shard heads (16) across 8 cores: 2 heads/core. Each core holds W_q/W_o
    column/row shards and its heads' K/V slices; partial outputs summed on host.
  - All matmul operands bf16 (full PE rate like fp32r, but half the DMA bytes
    and no small-N rate cliff); PSUM accumulation stays fp32.
  - S^T layout [m partitions, t free] so softmax weights feed P@V directly as
    matmul operands with no on-device transposes anywhere.
  - Queries sorted by position on host -> causal mask is a per-(m-tile) t-prefix;
    fully-masked columns are skipped via per-m-tile column windows (S/exp/PV/
    rowsum all windowed); the ~128-position partial band is masked with one
    scalar_tensor_tensor (pos >= m) * P per m-tile.
  - exp without max-subtraction (logits ~N(0,1); overflow impossible).
  - Row sums via ones-matmul on PE (replicated), reciprocal_approx_fast, fold
    into O^T normalize; out = W_o-shard projection in bf16, partials summed on
    host in fp64.
  - xT staged in DRAM as [128, t-half, ko, 512] so Q projection for the first
    t-half starts after ~2MB instead of the full 4.2MB.
"""
import sys
import math

import numpy as np

for _p in ("/opt/trn_rl_repo", "/opt/pypackages"):
    if _p not in sys.path:
        sys.path.append(_p)

import ml_dtypes

BF16 = ml_dtypes.bfloat16

T, D, H, DK, M = 1024, 2048, 16, 128, 4096
NCORES = 8
HLOC = H // NCORES  # heads per core
KO = D // 128       # 16 contraction tiles for projections
MT = M // 128       # 32 m-tiles
ROPE_BASE = 10000.0
SCALE = 1.0 / math.sqrt(DK)

_PROGRAM_CACHE = {}


def _host_rope_k(k, pos):
    """Apply RoPE to cached keys on host (fp64 tables). k: [M, h, DK]."""
    inv = 1.0 / (ROPE_BASE ** (np.arange(0, DK, 2, dtype=np.float64) / DK))
    th = pos[:, None].astype(np.float64) * inv[None, :]
    cos = np.concatenate([np.cos(th), np.cos(th)], -1)[:, None, :]
    sin = np.concatenate([np.sin(th), np.sin(th)], -1)[:, None, :]
    t1, t2 = k[..., :64], k[..., 64:]
    rot = np.concatenate([-t2, t1], -1)
    return (k.astype(np.float64) * cos + rot.astype(np.float64) * sin).astype(np.float32)


def _host_q_tables(pos_sorted):
    """cos / sign-baked sin tables in Q^T layout [DK, T] (fp32)."""
    inv = 1.0 / (ROPE_BASE ** (np.arange(0, DK, 2, dtype=np.float64) / DK))
    th = pos_sorted[None, :].astype(np.float64) * inv[:, None]      # [64, T]
    cos = np.cos(th)
    sin = np.sin(th)
    cosT = np.concatenate([cos, cos], 0).astype(np.float32)          # [128, T]
    sinT = np.concatenate([-sin, sin], 0).astype(np.float32)         # sign baked
    return cosT, sinT


RSG = 4  # m-tiles per rowsum group (P-tiles pre-summed on DVE, one ones-matmul)


def _windows(a_list, b_list):
    """Per m-tile: chunk column windows + exp/stt windows.

    Returns (tiles, groups). tiles[i] = (chunk_lo[2], exp_lo, stt_lo, stt_hi):
    chunk_lo[c] is the start column for S/PV matmuls in chunk c (None = skip);
    starts are 16-col aligned (32B for bf16 matmul operands). exp/stt start at
    the GROUP's min window so grouped P-tiles can be tree-summed over one
    region (the widened prefix is exactly zeroed by the stt mask).
    groups[g] = (glo_exp, rs_lo[2]): the group's exp window start and per-chunk
    rowsum matmul windows."""
    raw = []
    for i in range(MT):
        a = a_list[i]
        chunk_lo = []
        for c in range(2):
            lo_raw = max(512 * c, a)
            hi = 512 * (c + 1)
            if hi - lo_raw <= 0:
                chunk_lo.append(None)
                continue
            chunk_lo.append(max(512 * c, lo_raw & ~15))
        raw.append(chunk_lo)
    groups = []
    for g in range(MT // RSG):
        members = raw[g * RSG:(g + 1) * RSG]
        rs_lo = []
        for c in range(2):
            los = [m[c] for m in members if m[c] is not None]
            rs_lo.append(min(los) if los else None)
        glo_exp = next((rs_lo[c] for c in range(2) if rs_lo[c] is not None), None)
        groups.append((glo_exp, rs_lo))
    tiles = []
    for i in range(MT):
        chunk_lo = raw[i]
        glo_exp = groups[i // RSG][0]
        tiles.append((chunk_lo, glo_exp, glo_exp, b_list[i]))
    return tiles, groups


def _build_program(a_list, b_list):
    """Build the single-core Bass program (same for all cores)."""
    import concourse.tile as tile
    import concourse.mybir as mybir
    from concourse import bacc
    from contextlib import ExitStack

    f32 = mybir.dt.float32
    f32r = mybir.dt.float32r
    bf16 = mybir.dt.bfloat16
    win, grp = _windows(a_list, b_list)

    nc = bacc.Bacc("TRN2", target_bir_lowering=False, debug=False, num_devices=NCORES)

    d_xT = nc.dram_tensor("xT", (128, 2, KO, 512), bf16, kind="ExternalInput").ap()
    d_wqT = nc.dram_tensor("wqT", (128, KO, HLOC * DK), bf16, kind="ExternalInput").ap()
    d_ktr = nc.dram_tensor("ktr", (HLOC, DK, M), bf16, kind="ExternalInput").ap()
    d_v = nc.dram_tensor("v", (HLOC, 128, MT, DK), bf16, kind="ExternalInput").ap()
    d_woT = nc.dram_tensor("woT", (128, HLOC, D), bf16, kind="ExternalInput").ap()
    d_cosq = nc.dram_tensor("cosq", (DK, T), f32, kind="ExternalInput").ap()
    d_sinq = nc.dram_tensor("sinq", (DK, T), f32, kind="ExternalInput").ap()
    d_posr = nc.dram_tensor("posr", (128, T), f32r, kind="ExternalInput").ap()
    d_miota = nc.dram_tensor("miota", (128, MT), f32r, kind="ExternalInput").ap()
    d_ones = nc.dram_tensor("ones", (128, 128), bf16, kind="ExternalInput").ap()
    d_out = nc.dram_tensor("outT", (D, T), bf16, kind="ExternalOutput").ap()

    with tile.TileContext(nc) as tc, ExitStack() as ctx:
        const = ctx.enter_context(tc.tile_pool(name="const", bufs=1))
        big = ctx.enter_context(tc.tile_pool(name="big", bufs=1))
        qpool = ctx.enter_context(tc.tile_pool(name="qpool", bufs=2))
        qtmp = ctx.enter_context(tc.tile_pool(name="qtmp", bufs=2))
        ps_main = ctx.enter_context(tc.tile_pool(name="ps_main", bufs=2, space="PSUM"))
        ps_acc = ctx.enter_context(tc.tile_pool(name="ps_acc", bufs=1, space="PSUM"))
        ps_rs = ctx.enter_context(tc.tile_pool(name="ps_rs", bufs=1, space="PSUM"))
        xpool_cm = tc.tile_pool(name="xpool", bufs=1)
        xpool = xpool_cm.__enter__()

        # ---------------- loads ----------------
        # sync ring order IS the schedule: tiny mask tables first, then the
        # Qproj critical path (wqT, xT by t-half), then K/V/woT behind it.
        # Scalar ring carries only the rope tables so the ACT queue is never
        # blocked by slow DMA descriptor generation (e.g. posr's broadcast).
        ones_sb = const.tile([128, 128], bf16, name="ones_sb")
        nc.scalar.dma_start(out=ones_sb[:], in_=d_ones)
        wqT_sb = xpool.tile([128, KO, HLOC * DK], bf16, name="wqT_sb")
        nc.sync.dma_start(out=wqT_sb[:], in_=d_wqT)
        xT_sb = xpool.tile([128, 2, KO, 512], bf16, name="xT_sb")
        for c in range(2):
            for g in range(4):
                sl = slice(g * 4, (g + 1) * 4)
                nc.sync.dma_start(out=xT_sb[:, c, sl, :], in_=d_xT[:, c, sl, :])
        cosq_sb = const.tile([128, T], f32, name="cosq_sb")
        nc.scalar.dma_start(out=cosq_sb[:], in_=d_cosq)
        sinq_sb = const.tile([128, T], f32, name="sinq_sb")
        nc.scalar.dma_start(out=sinq_sb[:], in_=d_sinq)
        posr_sb = const.tile([128, T], f32r, name="posr_sb")
        nc.scalar.dma_start(out=posr_sb[:], in_=d_posr)
        miota_sb = const.tile([128, MT], f32r, name="miota_sb")
        nc.scalar.dma_start(out=miota_sb[:], in_=d_miota)

        # bulk loads behind xT on the sync ring: ktr0/v0 land just before the
        # first S/PV needs them
        ktr_sb = []
        v_sb = []
        for h in range(HLOC):
            kt = big.tile([128, M], bf16, name=f"ktr_sb{h}")
            nc.sync.dma_start(out=kt[:], in_=d_ktr[h])
            ktr_sb.append(kt)
            vt = big.tile([128, MT, DK], bf16, name=f"v_sb{h}")
            nc.sync.dma_start(out=vt[:], in_=d_v[h])
            v_sb.append(vt)
        woT_sb = big.tile([128, HLOC, D], bf16, name="woT_sb")
        nc.sync.dma_start(out=woT_sb[:], in_=d_woT)

        # dummy exp: forces the ACT exp-table load off the critical path
        # (it happens here, while ACT is otherwise idle during the loads)
        warm = const.tile([128, 16], bf16, name="warm")
        nc.scalar.activation(warm[:], ones_sb[:, 0:16],
                             mybir.ActivationFunctionType.Exp)

        # ---------------- Q projection + RoPE ----------------
        # h-outer: head 0's both chunks project+rope first, so the attention
        # stream (which needs full qtr[0]) starts as early as possible
        qps = []
        qtr = []
        for h in range(HLOC):
            # allocate Qproj accumulators from the attention accumulator pools
            # (idle during Qproj) so ps_main's S-buffers stay free
            qpool_ps = ps_acc if h == 0 else ps_rs
            qtag = "oacc" if h == 0 else "rs"
            qps.append(qpool_ps.tile([128, T], f32, tag=qtag, name=f"qps{h}"))
            qtr.append(qpool.tile([128, T], bf16, tag="qtr", name=f"qtr{h}"))
        # chain order: head 0's both chunks first (its rope gates the whole
        # attention stream); each rope is emitted right after its producing
        # chain — tile deps are tracked conservatively in program order, so
        # a rope emitted later would wait on unrelated writes to its qps tile
        for c, h in [(0, 0), (0, 1), (1, 0), (1, 1)]:
            cs = slice(c * 512, (c + 1) * 512)
            for k in range(KO):
                nc.tensor.matmul(
                    qps[h][:, cs],
                    wqT_sb[:, k, h * DK:(h + 1) * DK],
                    xT_sb[:, c, k, :],
                    start=(k == 0), stop=(k == KO - 1),
                )
            qrot = qtmp.tile([128, 512], f32, tag="qrot")
            nc.vector.tensor_copy(qrot[0:64, :], qps[h][64:128, cs])
            nc.vector.tensor_copy(qrot[64:128, :], qps[h][0:64, cs])
            t1 = qtmp.tile([128, 512], f32, tag="t1")
            nc.vector.tensor_mul(t1[:], qrot[:], sinq_sb[:, cs])
            t2 = qtmp.tile([128, 512], f32, tag="t2")
            nc.vector.tensor_mul(t2[:], qps[h][:, cs], cosq_sb[:, cs])
            nc.vector.tensor_add(qtr[h][:, cs], t1[:], t2[:])

        xpool_cm.__exit__(None, None, None)  # free xT/wqT SBUF for attention pools
        ppool = ctx.enter_context(tc.tile_pool(name="ppool", bufs=6))
        gpool = ctx.enter_context(tc.tile_pool(name="gpool", bufs=2))
        opool = ctx.enter_context(tc.tile_pool(name="opool", bufs=2))
        ostage = ctx.enter_context(tc.tile_pool(name="ostage", bufs=3))

        # ---------------- attention per head ----------------
        # PE issue order is software-pipelined: S(i+2) is issued right after
        # PV(i) so exp(i+1) never waits behind PV(i) in the in-order PE queue;
        # rowsum group matmuls are deferred ~2 tiles so their DVE tree-adds
        # are complete by the time the PE reaches them.
        onorm = []
        ptiles = {}

        def emit_s_exp(h, i):
            chunk_lo, exp_lo, stt_lo, stt_hi = win[i]
            s_lo = grp[i // RSG][1]  # group windows: exp reads only S-written psum
            sps = ps_main.tile([128, T], f32, tag="mm", name=f"s_{h}_{i}")
            for c in range(2):
                lo = s_lo[c]
                if lo is None:
                    continue
                nc.tensor.matmul(
                    sps[:, lo:512 * (c + 1)],
                    ktr_sb[h][:, i * 128:(i + 1) * 128],
                    qtr[h][:, lo:512 * (c + 1)],
                    start=True, stop=True,
                )
            p = ppool.tile([128, T], bf16, tag="p")
            nc.scalar.activation(p[:, exp_lo:], sps[:, exp_lo:],
                                 mybir.ActivationFunctionType.Exp, scale=SCALE)
            if stt_hi > stt_lo:
                nc.vector.scalar_tensor_tensor(
                    out=p[:, stt_lo:stt_hi], in0=posr_sb[:, stt_lo:stt_hi],
                    scalar=miota_sb[:, i:i + 1], in1=p[:, stt_lo:stt_hi],
                    op0=mybir.AluOpType.is_ge, op1=mybir.AluOpType.mult,
                )
            ptiles[(h, i)] = p

        # who issues the final (stop) rowsum write per chunk: a direct matmul
        # in the last group if that chunk exists there, else the last tree group
        last_writer = {}
        for c in range(2):
            lg = max((g for g in range(MT // RSG) if grp[g][1][c] is not None),
                     default=None)
            if lg == MT // RSG - 1:
                lk = max(k for k in range(RSG)
                         if win[lg * RSG + k][0][c] is not None)
                last_writer[c] = ("direct", lk)
            else:
                last_writer[c] = ("tree", lg)

        emit_s_exp(0, 0)
        emit_s_exp(0, 1)
        for h in range(HLOC):
            ops_t = ps_acc.tile([128, T], f32, tag="oacc", name=f"oacc{h}")
            rs_t = ps_rs.tile([128, T], f32, tag="rs", name=f"rs{h}")
            started = [False, False]
            rs_started = [False, False]
            gsums = {}
            pending_rs = []
            for i in range(MT):
                chunk_lo, exp_lo, stt_lo, stt_hi = win[i]
                p = ptiles[(h, i)]
                for c in range(2):
                    lo = chunk_lo[c]
                    if lo is None:
                        continue
                    nc.tensor.matmul(
                        ops_t[:, lo:512 * (c + 1)],
                        v_sb[h][:, i, :],
                        p[:, lo:512 * (c + 1)],
                        start=not started[c], stop=(i == MT - 1),
                    )
                    started[c] = True
                if i % RSG == RSG - 1 and i // RSG == MT // RSG - 1:
                    # last group: feed rowsums directly from the P tiles (no
                    # tree) so the normalize isn't gated on trailing DVE adds
                    g = i // RSG
                    for k in range(RSG):
                        tk = g * RSG + k
                        k_lo = win[tk][0]
                        for c in range(2):
                            lo = k_lo[c]
                            if lo is None:
                                continue
                            nc.tensor.matmul(
                                rs_t[:, lo:512 * (c + 1)],
                                ones_sb[:],
                                ptiles[(h, tk)][:, lo:512 * (c + 1)],
                                start=not rs_started[c],
                                stop=(last_writer[c] == ("direct", k)),
                            )
                            rs_started[c] = True
                elif i % RSG == RSG - 1:
                    # tree-sum the group's P tiles on DVE (bf16, SBUF->SBUF)
                    g = i // RSG
                    glo, _ = grp[g]
                    pg = [ptiles[(h, g * RSG + k)] for k in range(RSG)]
                    t01 = gpool.tile([128, T], bf16, tag="t01")
                    nc.vector.tensor_add(t01[:, glo:], pg[0][:, glo:], pg[1][:, glo:])
                    t23 = gpool.tile([128, T], bf16, tag="t23")
                    nc.vector.tensor_add(t23[:, glo:], pg[2][:, glo:], pg[3][:, glo:])
                    psum_g = gpool.tile([128, T], bf16, tag="psum_g")
                    nc.vector.tensor_add(psum_g[:, glo:], t01[:, glo:], t23[:, glo:])
                    gsums[g] = psum_g
                    pending_rs.append(g)
                # deferred rowsum matmuls: group g is safe once PV(4g+5) issued
                while pending_rs and 4 * pending_rs[0] + 5 <= i:
                    g = pending_rs.pop(0)
                    _, rs_lo = grp[g]
                    for c in range(2):
                        lo = rs_lo[c]
                        if lo is None:
                            continue
                        nc.tensor.matmul(
                            rs_t[:, lo:512 * (c + 1)],
                            ones_sb[:],
                            gsums[g][:, lo:512 * (c + 1)],
                            start=not rs_started[c],
                            stop=(last_writer[c] == ("tree", g)),
                        )
                        rs_started[c] = True
                # pipeline 2 S/exp tiles ahead, crossing the head boundary so
                # the next head's exp stream starts before this head's drain
                ni = i + 2
                if ni < MT:
                    emit_s_exp(h, ni)
                elif h + 1 < HLOC:
                    emit_s_exp(h + 1, ni - MT)
            for g in pending_rs:
                _, rs_lo = grp[g]
                for c in range(2):
                    lo = rs_lo[c]
                    if lo is None:
                        continue
                    nc.tensor.matmul(
                        rs_t[:, lo:512 * (c + 1)],
                        ones_sb[:],
                        gsums[g][:, lo:512 * (c + 1)],
                        start=not rs_started[c],
                        stop=(last_writer[c] == ("tree", g)),
                    )
                    rs_started[c] = True
            oh = opool.tile([128, T], bf16, tag=f"onorm{h}", name=f"onorm{h}")
            onorm.append(oh)
            if h == 0:
                for c in range(2):
                    cs = slice(c * 512, (c + 1) * 512)
                    rsinv = qtmp.tile([128, 512], f32, tag="rsinv")
                    nc.vector.reciprocal_approx_fast(out=rsinv[:], in_=rs_t[:, cs])
                    nc.vector.tensor_mul(oh[:, cs], ops_t[:, cs], rsinv[:])
            else:
                last_ops, last_rs = ops_t, rs_t

        # ---------------- output projection ----------------
        # head-0 contributions for j=0,1 are issued (in program order) BEFORE
        # head 1's normalize: they only need onorm[0] (ready mid-kernel), so
        # the PE stays busy while the normalize runs on DVE (avoids an idle
        # gap + p-state drop)
        outT_r = d_out.rearrange("(jo p) t -> p jo t", p=128)
        jtiles = {}
        for j in range(2):
            jps = ps_main.tile([128, T], f32, tag="mm", name=f"jps{j}")
            jtiles[j] = jps
            for c in range(2):
                nc.tensor.matmul(
                    jps[:, c * 512:(c + 1) * 512],
                    woT_sb[:, 0, j * 128:(j + 1) * 128],
                    onorm[0][:, c * 512:(c + 1) * 512],
                    start=True, stop=False,
                )
        for c in range(2):
            cs = slice(c * 512, (c + 1) * 512)
            rsinv = qtmp.tile([128, 512], f32, tag="rsinv")
            nc.vector.reciprocal_approx_fast(out=rsinv[:], in_=last_rs[:, cs])
            nc.vector.tensor_mul(onorm[1][:, cs], last_ops[:, cs], rsinv[:])

        for j in range(KO):
            if j in jtiles:
                jps = jtiles[j]
                hos = [1]
            elif j % 4 == 2:
                jps = ps_acc.tile([128, T], f32, tag="oacc", name=f"jps{j}")
                hos = [0, 1]
            elif j % 4 == 3:
                jps = ps_rs.tile([128, T], f32, tag="rs", name=f"jps{j}")
                hos = [0, 1]
            else:
                jps = ps_main.tile([128, T], f32, tag="mm", name=f"jps{j}")
                hos = [0, 1]
            for ho in hos:
                for c in range(2):
                    nc.tensor.matmul(
                        jps[:, c * 512:(c + 1) * 512],
                        woT_sb[:, ho, j * 128:(j + 1) * 128],
                        onorm[ho][:, c * 512:(c + 1) * 512],
                        start=(ho == 0 and j not in jtiles), stop=(ho == HLOC - 1),
                    )
            ost = ostage.tile([128, T], bf16, tag="ost")
            if j >= KO - 2:
                # drain: split the copy across both engines to halve latency
                nc.vector.tensor_copy(ost[:, 0:512], jps[:, 0:512])
                nc.scalar.copy(ost[:, 512:1024], jps[:, 512:1024])
                nc.sync.dma_start(out=outT_r[:, j, 0:512], in_=ost[:, 0:512])
                nc.scalar.dma_start(out=outT_r[:, j, 512:1024], in_=ost[:, 512:1024])
            elif j % 2 == 0:
                nc.vector.tensor_copy(ost[:], jps[:])
                nc.sync.dma_start(out=outT_r[:, j, :], in_=ost[:])
            else:
                nc.scalar.copy(ost[:], jps[:])
                nc.scalar.dma_start(out=outT_r[:, j, :], in_=ost[:])

    nc.compile()
    return nc


def _prep(inputs):
    """Host-side prep shared by kernel() and test harnesses."""
    x = np.asarray(inputs["x"], dtype=np.float32)
    k_ctx = np.asarray(inputs["k_ctx"], dtype=np.float32)
    v_ctx = np.asarray(inputs["v_ctx"], dtype=np.float32)
    W_q = np.asarray(inputs["W_q"], dtype=np.float32)
    W_o = np.asarray(inputs["W_o"], dtype=np.float32)
    pos_np = np.asarray(inputs["positions"]).astype(np.int64)
    pctx_np = np.asarray(inputs["p_ctx"]).astype(np.int64)

    perm = np.argsort(pos_np, kind="stable")
    ps = pos_np[perm]
    xT = np.ascontiguousarray(
        x[perm].T.reshape(KO, 128, 2, 512).transpose(1, 2, 0, 3)).astype(BF16)
    k_rope = _host_rope_k(k_ctx, pctx_np)
    cosq, sinq = _host_q_tables(ps)
    posr = np.ascontiguousarray(
        np.broadcast_to(ps.astype(np.float32).reshape(1, T), (128, T)))
    miota = (np.arange(MT)[None, :] * 128 + np.arange(128)[:, None]).astype(np.float32)
    ones = np.ones((128, 128), dtype=BF16)
    a_list = [int(np.searchsorted(ps, 128 * i, side="left")) for i in range(MT)]
    b_list = [int(np.searchsorted(ps, 128 * i + 127, side="left")) for i in range(MT)]

    in_maps = []
    for c in range(NCORES):
        hs = slice(c * HLOC * DK, (c + 1) * HLOC * DK)
        heads = range(c * HLOC, (c + 1) * HLOC)
        wq = W_q[hs, :].T.reshape(KO, 128, HLOC * DK)          # [ko, p, o]
        wo = W_o[:, hs].T.reshape(HLOC, 128, D)                 # [ho, p, j]
        vv = v_ctx.transpose(1, 0, 2)[c * HLOC:(c + 1) * HLOC]  # [hloc, M, DK]
        in_maps.append({
            "xT": xT,
            "wqT": np.ascontiguousarray(wq.transpose(1, 0, 2)).astype(BF16),
            "ktr": np.ascontiguousarray(
                np.stack([k_rope[:, h, :].T for h in heads])).astype(BF16),
            "v": np.ascontiguousarray(
                vv.reshape(HLOC, MT, 128, DK).transpose(0, 2, 1, 3)).astype(BF16),
            "woT": np.ascontiguousarray(wo.transpose(1, 0, 2)).astype(BF16),
            "cosq": cosq, "sinq": sinq, "posr": posr,
            "miota": miota, "ones": ones,
        })
    return perm, a_list, b_list, in_maps


def kernel(x, k_ctx, v_ctx, W_q, W_o, positions, p_ctx):
    from concourse.bass_utils import run_bass_kernel_spmd

    inputs = dict(x=x, k_ctx=k_ctx, v_ctx=v_ctx, W_q=W_q, W_o=W_o,
                  positions=positions, p_ctx=p_ctx)
    perm, a_list, b_list, in_maps = _prep(inputs)

    key = (tuple(a_list), tuple(b_list))
    if key not in _PROGRAM_CACHE:
        _PROGRAM_CACHE[key] = _build_program(a_list, b_list)
    nc = _PROGRAM_CACHE[key]

    r = run_bass_kernel_spmd(nc, in_maps, core_ids=list(range(NCORES)))

    acc = np.zeros((D, T), dtype=np.float64)
    for c in range(NCORES):
        acc += r.results[c]["outT"].astype(np.float64)
    out_sorted = acc.T.astype(np.float32)
    out = np.empty_like(out_sorted)
    out[perm] = out_sorted
    return out.astype(np.float32)


if __name__ == "__main__":
    import importlib.util
    spec = importlib.util.spec_from_file_location("reference", "/root/problem/reference.py")
    ref = importlib.util.module_from_spec(spec)
    spec.loader.exec_module(ref)
    inputs = {k: np.asarray(v) for k, v in ref.setup_inputs().items()}
    expected = np.asarray(ref.reference(**inputs))
    got = kernel(**inputs)
    err = np.abs(got - expected)
    print("absmax err:", err.max(), "rel:", err.max() / np.abs(expected).max())
